# revision 55
# baseline (speedup 1.0000x reference)
"""Position-attention (SAGAN-style) Bass kernel for 8 Trainium2 NeuronCores.

Reference computation (per batch b, with n = H*W = 4096 spatial tokens):
    q = Wq @ x + bq            [32, n]
    k = Wk @ x + bk            [32, n]
    v = Wv @ x + bv            [256, n]
    att = softmax_j(q_i . k_j) [n, n]
    out = gamma * (v @ att^T) + x

Sharding: 8 cores = 4 batches x 2 token-halves; disjoint outputs, no
collectives. SPMD-uniform program: the host permutes each core's x so its
own 2048 tokens are columns [0:2048) (attention is permutation-invariant
over j, and out/q only touch own columns).

Layout/engine choices:
  - scores^T[j, i] = lhsT(k[d, j_tile]) . rhs(q[d, i]): K=32 contraction,
    4-way PE row-tiling (tile_position=(32t, 0)); q/k built 4x-replicated
    across partitions so row group t finds operands at base partition 32t.
  - rowsum[i] = sum_j e^T[j, i]: M=1 ones-matmuls, 4-way PE column-tiling
    (tile_position=(0, 32c)) -> 4 partials at partitions {0,32,64,96} of
    one PSUM bank. A leading zero-matmul (lhsT=0, M=128) opens the bank:
    one whole-bank has_written clear + zeroes garbage partitions, so the
    4 col-groups can all accumulate with start=False and an all-ones
    K=128 matmul later does combine+broadcast in one shot.
  - 1/rowsum via exp(-ln(rs)) on ScalarE (both funcs live in the
    natural_log_exp_and_others table set -> one ACT_TABLE_LOAD); the DVE
    reciprocal is ~6.3 cyc/elem and was 13.4us of Vector time.
  - out[c, i] = sum_j v^T[j, c] e^T[j, i]: K=128 bf16 matmuls accumulated
    over 32 j-tiles in PSUM (fp8 was evaluated and rejected: quantizing
    e/v to any fp8 format costs 4-7e-2 rel err vs the 2e-2 budget).
  - x loaded once (4MB fp32), split across both HWDGE rings (sync +
    scalar queues), own-block0 chunk first so q/scores start early; bf16
    casts chase the DMAs. No separate xq load (was +2MB).
  - softmax needs no max-subtraction: max score ~25 << 88 (fp32 exp
    overflow), e in bf16. exp(-ln(rs + 1e-30)) keeps gamma=0 exact and
    degenerate rows finite.
Matmul operands are bf16 (fp32 PSUM accumulation); the residual `+ x` is
added in exact fp32.
"""

import os
import sys

for _p in (
    "/root/.axon_site",
    "/root/.axon_site/_ro/trn_rl_repo",
    "/root/.axon_site/_ro/pypackages",
    "/opt/trn_rl_repo",
):
    if os.path.isdir(_p) and _p not in sys.path:
        sys.path.append(_p)

import json

import numpy as np

from concourse import bass, mybir
from concourse.tile import TileContext

F32 = mybir.dt.float32
BF16 = mybir.dt.bfloat16

B, C, H, W = 4, 256, 64, 64
N = H * W            # 4096 tokens
NH = N // 2          # 2048 tokens per core (token half)
MID = C // 8         # 32 qk channels
JT = N // 128        # 32 j-tiles of 128 tokens
NBLK = NH // 512     # 4 i-blocks of 512 tokens per core

SCORE_TP = 4         # score matmul row-tiling ways (4 -> positions 0/32/64/96)
RS_COLS = 4          # rowsum matmul column-tiling ways


def _split_multi_waits(bir_bytes: bytes) -> bytes:
    """Workaround for this container's walrus: it accepts at most ONE sem-wait
    command per lowered instruction ('Too many sync wait commands'), while
    bass/Tile freely attach several. Split extra waits onto preceding NoOps
    on the same engine — per-engine program order makes this semantics-
    preserving (all waits still satisfied before the instruction runs)."""
    d = json.loads(bir_bytes)
    n_split = 0
    for f in d.get("functions", []):
        for bb in f.get("blocks", []):
            out = []
            for ins in bb.get("instructions", []):
                si = ins.get("sync_info")
                waits = si.get("on_wait") if si else None
                if waits and len(waits) > 1:
                    for w in waits[:-1]:
                        n_split += 1
                        out.append(
                            {
                                "debug": ins.get("debug", 0),
                                "engine": ins["engine"],
                                "ins": [],
                                "outs": [],
                                "name": f"{ins['name']}-ws{n_split}",
                                "opcode": "NoOp",
                                "sync_info": {"on_wait": [w], "on_update": []},
                            }
                        )
                    si["on_wait"] = [waits[-1]]
                out.append(ins)
            bb["instructions"] = out
    return json.dumps(d).encode()


_ws_applied = False


def _apply_wait_split_patch():
    global _ws_applied
    if _ws_applied:
        return
    _ws_applied = True
    from concourse import bass_utils, bass2jax

    orig = bass_utils.compile_bir_kernel

    def patched(bir_json, tmpdir, neff_name="file.neff"):
        return orig(_split_multi_waits(bytes(bir_json)), tmpdir, neff_name)

    bass_utils.compile_bir_kernel = patched
    bass2jax.compile_bir_kernel = patched


_apply_wait_split_patch()


def _build_program():
    nc = bass.Bass()

    xf_d = nc.declare_dram_parameter("xf", [C, N], F32, isOutput=False)
    wT_d = nc.declare_dram_parameter("wT", [C, 512], F32, isOutput=False)
    # [bq4 | bk4 | g128 | bvP] packed: one DMA instead of four (each tiny
    # DMA costs ~2us of serial ring time)
    cst_d = nc.declare_dram_parameter("cst", [128, 5], F32, isOutput=False)
    out_d = nc.declare_dram_parameter("out", [C, NH], F32, isOutput=True)

    act = mybir.ActivationFunctionType
    add = mybir.AluOpType.add

    with TileContext(nc) as tc:
        with (
            tc.tile_pool(name="const", bufs=1) as constp,
            tc.tile_pool(name="xf", bufs=1) as xfp,
            tc.tile_pool(name="xb", bufs=1) as xbp,
            tc.tile_pool(name="proj", bufs=1) as projp,
            tc.tile_pool(name="eblk", bufs=2) as eblkp,
            tc.tile_pool(name="small", bufs=2) as smallp,
            tc.tile_pool(name="res", bufs=4) as resp,
            tc.tile_pool(name="psA", bufs=2, space="PSUM") as psA,
            tc.tile_pool(name="psB", bufs=2, space="PSUM") as psB,
            tc.tile_pool(name="psRS", bufs=1, space="PSUM") as psRS,
            tc.tile_pool(name="psBC", bufs=1, space="PSUM") as psBC,
        ):
            # ---- constants / weights ----
            # w split across both HWDGE rings: the q/k half gates the
            # first projections, the v half only the v-projs. Small
            # consts ride the sync ring after its x pieces.
            cst = constp.tile([128, 5], F32, tag="cst")
            nc.gpsimd.dma_start(out=cst[:, :], in_=cst_d[:, :])
            bq4, bk4, g128, bvP = (cst[:, 0:1], cst[:, 1:2], cst[:, 2:3],
                                   cst[:, 3:5])

            w_f = constp.tile([128, 2, 512], F32, tag="wf")
            nc.scalar.dma_start(
                out=w_f[:, :, 0:256],
                in_=wT_d[:, 0:256].rearrange("(two p) n -> p two n",
                                             two=2))
            nc.scalar.dma_start(
                out=w_f[:, :, 256:512],
                in_=wT_d[:, 256:512].rearrange("(two p) n -> p two n",
                                               two=2))
            w_b = constp.tile([128, 2, 512], BF16, tag="wb")
            nc.vector.tensor_copy(w_b[:, :, 0:256], w_f[:, :, 0:256])

            ones_b = constp.tile([128, 1], BF16, tag="ones_b")
            nc.vector.memset(ones_b[:, :], 1.0)
            ones_f = constp.tile([128, 128], F32, tag="ones_f")
            nc.vector.memset(ones_f[:, :], 1.0)
            zero_b = constp.tile([128, 128], BF16, tag="zero_b")
            nc.vector.memset(zero_b[:, :], 0.0)
            eps128 = constp.tile([128, 1], F32, tag="eps")
            nc.vector.memset(eps128[:, :], 1e-30)

            # ---- x loads: own-block0 small chunk first (unblocks q and the
            # first score group), own-rest on the sync ring, other half on
            # the scalar ring so the two HWDGE rings transfer in parallel ----
            xf_f = xfp.tile([128, 2, N], F32, tag="xff")
            x_b = xbp.tile([128, 2, N], BF16, tag="xb")
            pieces = [  # (c0, c1, engine) — pieces 0-1 alone on the sync
                # ring and w alone ahead on the scalar ring, so the
                # critical first transfers don't share HBM bandwidth;
                # later pieces queue behind w on the scalar ring
                (0, 512, nc.sync),
                (512, 1024, nc.sync),
                (1024, 1536, nc.scalar),
                (1536, 2048, nc.scalar),
                (2048, 3072, nc.scalar),
                (3072, 4096, nc.scalar),
            ]
            for c0, c1, eng in pieces:
                eng.dma_start(
                    out=xf_f[:, :, c0:c1],
                    in_=xf_d[:, c0:c1].rearrange("(two p) n -> p two n",
                                                 two=2))
            gb = constp.tile([128, 2], F32, tag="gb")

            def cast_piece(i):
                c0, c1, _ = pieces[i]
                for s0 in range(c0, c1, 512):
                    nc.vector.tensor_copy(x_b[:, :, s0:s0 + 512],
                                          xf_f[:, :, s0:s0 + 512])

            # ---- projections ----
            # q (4x-replicated rows): [128, NH]; own columns only
            q_sb = projp.tile([128, NH], BF16, tag="q")

            def q_proj(ic, pool, tag):
                ps = pool.tile([128, 512], F32, tag=tag, name=f"qp{ic}")
                nc.tensor.matmul(
                    ps[:, :], lhsT=w_b[:, 0, 0:128],
                    rhs=x_b[:, 0, ic * 512:(ic + 1) * 512],
                    start=True, stop=False)
                nc.tensor.matmul(
                    ps[:, :], lhsT=w_b[:, 1, 0:128],
                    rhs=x_b[:, 1, ic * 512:(ic + 1) * 512],
                    start=False, stop=True)
                if ic == 0:
                    # ScalarE is idle before the first exp; evacuating the
                    # first q/k chunks there skips the DVE-queue delay
                    # that otherwise gates scores(0,0)
                    nc.scalar.activation(
                        q_sb[:, 0:512], ps[:, :],
                        mybir.ActivationFunctionType.Identity,
                        bias=bq4[:, :])
                else:
                    nc.vector.tensor_scalar_add(
                        q_sb[:, ic * 512:(ic + 1) * 512], ps[:, :],
                        bq4[:, :])

            # k (4x-replicated rows): [128, N]
            k_sb = projp.tile([128, N], BF16, tag="k")

            def k_proj(ic, pool, tag):
                ps = pool.tile([128, 512], F32, tag=tag, name=f"kp{ic}")
                nc.tensor.matmul(
                    ps[:, :], lhsT=w_b[:, 0, 128:256],
                    rhs=x_b[:, 0, ic * 512:(ic + 1) * 512],
                    start=True, stop=False)
                nc.tensor.matmul(
                    ps[:, :], lhsT=w_b[:, 1, 128:256],
                    rhs=x_b[:, 1, ic * 512:(ic + 1) * 512],
                    start=False, stop=True)
                if ic == 0:
                    nc.scalar.activation(
                        k_sb[:, 0:512], ps[:, :],
                        mybir.ActivationFunctionType.Identity,
                        bias=bk4[:, :])
                else:
                    nc.vector.tensor_scalar_add(
                        k_sb[:, ic * 512:(ic + 1) * 512], ps[:, :],
                        bk4[:, :])

            # v^T tiles, flat [128, JT*256]: tile jt at cols [jt*256, +256);
            # two tiles share one PSUM bank so the evacuating cast is a
            # single [128, 512] copy. bv folds into the epilogue.
            v_sb = projp.tile([128, JT * C], BF16, tag="v")

            def v_proj(p, pool, tag):
                ps = pool.tile([128, 512], F32, tag=tag, name=f"vp{p}")
                for t in range(2):
                    jt = 2 * p + t
                    nc.tensor.matmul(
                        ps[:, t * 256:t * 256 + C],
                        lhsT=x_b[:, 0, jt * 128:(jt + 1) * 128],
                        rhs=w_b[:, 0, 256:512], start=True, stop=False)
                    nc.tensor.matmul(
                        ps[:, t * 256:t * 256 + C],
                        lhsT=x_b[:, 1, jt * 128:(jt + 1) * 128],
                        rhs=w_b[:, 1, 256:512], start=False, stop=True)
                nc.vector.tensor_copy(v_sb[:, p * 512:(p + 1) * 512],
                                      ps[:, :])

            # Projection emission plan: the first pieces' q/k go up front
            # through psB, then block 0 opens immediately (its exp stream
            # is the critical path); the remaining projections dribble
            # through BOTH psB (free until block 0's out-accumulators are
            # allocated at group 5) and the psBC bank, in parallel chains,
            # scheduled ahead of their first use. x-casts are emitted just
            # before their first consumer so the in-order DVE queue never
            # parks on a late DMA.
            def pjB(fn, a):
                return lambda: fn(a, psB, "psb")

            def pjC(fn, a):
                return lambda: fn(a, psBC, "psbc")

            proj_sched = {
                0: [lambda: cast_piece(2), pjB(k_proj, 2), pjC(v_proj, 4),
                    pjB(v_proj, 5)],
                1: [lambda: cast_piece(3), pjC(k_proj, 3), pjB(v_proj, 6),
                    pjC(v_proj, 7)],
                2: [lambda: cast_piece(4), pjB(k_proj, 4), pjC(q_proj, 2),
                    pjB(v_proj, 8), pjC(v_proj, 9)],
                3: [lambda: cast_piece(5), pjB(k_proj, 5), pjC(q_proj, 3),
                    pjB(v_proj, 10), pjC(v_proj, 11)],
                4: [pjB(k_proj, 6), pjC(v_proj, 12), pjB(v_proj, 13)],
                5: [pjC(k_proj, 7), pjB(v_proj, 14), pjC(v_proj, 15)],
            }

            # ---- attention blocks: 4 i-blocks of 512 tokens ----
            # Phases fused and software-pipelined: the PE-queue order per
            # group is [scores(g+1) | rowsum(g) | out-acc(g)], so exp(g+1)
            # (ScalarE) overlaps the PE work on group g, and the last
            # block's tail is just one epilogue.
            NG = JT // SCORE_TP
            EB, RS, AC, ASB, RG = {}, {}, {}, {}, {}  # per-block state

            def scores(b, g):
                # one 4-way row-tiled group split across TWO 2-bank psA
                # allocations, so exp(g) reads one pair while the next
                # group's matmuls fill the other: the exp->scores->exp
                # serial chain (psA reuse) never stalls either engine
                i0 = b * 512
                halves = []
                for h in range(2):
                    ph = psA.tile([128, 2, 512], F32, tag="psa",
                                  name=f"ps4_{b}_{g}_{h}")
                    for u in range(2):
                        t = 2 * h + u
                        jt = SCORE_TP * g + t
                        nc.tensor.matmul(
                            ph[:, u, :],
                            lhsT=k_sb[32 * t:32 * (t + 1),
                                      jt * 128:(jt + 1) * 128],
                            rhs=q_sb[32 * t:32 * (t + 1), i0:i0 + 512],
                            start=True, stop=True,
                            tile_position=(32 * t, 0))
                    halves.append(ph)
                return halves

            def emit_exp(e_blk, g, ps4):
                for h, ph in enumerate(ps4):
                    j0 = SCORE_TP * g + 2 * h
                    nc.scalar.activation(
                        e_blk[:, j0:j0 + 2, :], ph[:, :, :], act.Exp)

            def out_group(b, g):
                for ch in range(2):
                    for t in range(SCORE_TP):
                        jt = SCORE_TP * g + t
                        nc.tensor.matmul(
                            AC[b][ch][:, :],
                            lhsT=v_sb[:, jt * 256 + ch * 128:
                                      jt * 256 + ch * 128 + 128],
                            rhs=EB[b][:, jt, :],
                            start=(jt == 0), stop=(jt == JT - 1))

            # out-matmul lag: block 0 defers its out-groups to iterations
            # 5..7 (so psB keeps pipelining projections until then); later
            # blocks lag by one group, which both absorbs the exp->PE
            # semaphore latency and gives the epilogue time to recycle the
            # psB banks across block boundaries.
            B0_OUT = {6: (0, 1, 2), 7: (3, 4, 5, 6)}

            cast_piece(0)
            q_proj(0, psB, "psb")
            k_proj(0, psB, "psb")
            EB[0] = eblkp.tile([128, JT, 512], BF16, tag="e", name="e0")
            ps4 = scores(0, 0)
            emit_exp(EB[0], 0, ps4)
            exp_emitted = True
            nc.vector.tensor_copy(w_b[:, :, 256:512], w_f[:, :, 256:512])
            # gb[c] = gamma * bv[c]  (folded v-bias: out += gamma*bv[c])
            nc.vector.tensor_scalar_mul(gb[:, :], bvP[:, :], g128[:, :])
            v_proj(0, psB, "psb")
            v_proj(1, psB, "psb")
            cast_piece(1)
            k_proj(1, psB, "psb")
            v_proj(2, psB, "psb")
            v_proj(3, psB, "psb")
            q_proj(1, psB, "psb")

            for idx in range(NBLK * NG):
                b, g = divmod(idx, NG)
                e_blk = EB[b]
                if g == 0:
                    # rowsum bank + bank-opener: one whole-bank
                    # has_written clear + zeroes, so the col-tiled
                    # partials all run start=False and the combine matmul
                    # can sum all 128 partitions. Emitted here (not with
                    # the early scores/exp) so it never delays them.
                    rs_ps = RS[b] = psRS.tile([128, 512], F32, tag="psrs",
                                              name=f"rs{b}")
                    nc.tensor.matmul(
                        rs_ps[:, :], lhsT=zero_b[:, :],
                        rhs=x_b[:, 0, 0:512],
                        start=True, stop=False, skip_group_check=True)
                    if b > 0:
                        # previous block's last out-group + accumulator
                        # evacuation + epilogue land HERE, giving the PE
                        # ready work while exp(b,0) runs (kills the
                        # block-boundary bubble)
                        out_group(b - 1, NG - 1)
                        ASB[b - 1] = []
                        for ch in range(2):
                            asb = resp.tile([128, 512], F32, tag="res",
                                            name=f"res{b - 1}_{ch}")
                            nc.vector.tensor_copy(asb[:, :],
                                                  AC[b - 1][ch][:, :])
                            ASB[b - 1].append(asb)
                        AC[b] = [psB.tile([128, 512], F32, tag="psb",
                                          name=f"acc{b}_{ch}")
                                 for ch in range(2)]
                        pi0 = (b - 1) * 512
                        for ch in range(2):
                            res = ASB[b - 1][ch]
                            nc.vector.tensor_mul(res[:, :], res[:, :],
                                                 RG[b - 1][:, :])
                            nc.vector.scalar_tensor_tensor(
                                res[:, :], res[:, :], gb[:, ch:ch + 1],
                                xf_f[:, ch, pi0:pi0 + 512],
                                op0=add, op1=add)
                            nc.sync.dma_start(
                                out=out_d[ch * 128:(ch + 1) * 128,
                                          pi0:pi0 + 512],
                                in_=res[:, :])
                if b == 0 and g == 6:
                    AC[0] = [psB.tile([128, 512], F32, tag="psb",
                                      name=f"acc0_{ch}")
                             for ch in range(2)]
                rs_ps = RS[b]
                if not exp_emitted:
                    emit_exp(e_blk, g, ps4)
                exp_emitted = False
                if g + 1 < NG:
                    ps4 = scores(b, g + 1)
                elif b + 1 < NBLK:
                    # next block's first scores+exp, ahead of this
                    # group's rowsum/out work: ScalarE flows from exp(b,7)
                    # straight into exp(b+1,0) while the PE catches up
                    EB[b + 1] = eblkp.tile([128, JT, 512], BF16, tag="e",
                                           name=f"e{b + 1}")
                    ps4 = scores(b + 1, 0)
                    emit_exp(EB[b + 1], 0, ps4)
                    exp_emitted = True
                for t in range(SCORE_TP):
                    jt = SCORE_TP * g + t
                    c = jt % RS_COLS
                    nc.tensor.matmul(
                        rs_ps[32 * c:32 * c + 1, :],
                        lhsT=ones_b[:, :], rhs=e_blk[:, jt, :],
                        start=False, stop=(jt >= JT - RS_COLS),
                        tile_position=(0, 32 * c),
                        skip_group_check=True)
                if b == 0:
                    for og in B0_OUT.get(g, ()):
                        out_group(0, og)
                    for job in proj_sched.get(g, ()):
                        job()
                elif g >= 1:
                    out_group(b, g - 1)
                if g < NG - 1:
                    continue

                # block end: rowsum partials -> SBUF; all-ones K=128
                # matmul does combine + broadcast to 128 partitions in one
                # shot; 1/rs as exp(-ln(rs)) on ScalarE (both funcs share
                # one ACT table set); * gamma on DVE. The accumulators are
                # evacuated to SBUF immediately (no rg dependency) so the
                # psB banks recycle for block b+1 without waiting on the
                # normalization chain.
                i0 = b * 512
                rs_sb = smallp.tile([128, 512], F32, tag="rs",
                                    name=f"rssb{b}")
                nc.vector.tensor_copy(rs_sb[:, :], rs_ps[:, :])
                bc_ps = psBC.tile([128, 512], F32, tag="psbc",
                                  name=f"bc{b}")
                nc.tensor.matmul(
                    bc_ps[:, :], lhsT=ones_f[:, :], rhs=rs_sb[:, :],
                    start=True, stop=True)
                last = b == NBLK - 1
                if last:
                    out_group(b, NG - 1)
                ln_sb = smallp.tile([128, 512], F32, tag="ln",
                                    name=f"ln{b}")
                nc.scalar.activation(ln_sb[:, :], bc_ps[:, :], act.Ln,
                                     bias=eps128[:, :])
                inv_sb = smallp.tile([128, 512], F32, tag="inv",
                                     name=f"inv{b}")
                nc.scalar.activation(inv_sb[:, :], ln_sb[:, :], act.Exp,
                                     scale=-1.0)
                rg = smallp.tile([128, 512], F32, tag="rg",
                                 name=f"rg{b}")
                nc.vector.tensor_scalar_mul(rg[:, :], inv_sb[:, :],
                                            g128[:, :])
                RG[b] = rg

                # non-last blocks defer their last out-group + epilogue to
                # the next block's first iteration (boundary filler); the
                # last block runs its epilogue straight from PSUM with the
                # two stores on parallel rings to shorten the tail
                if last:
                    for ch in range(2):
                        res = resp.tile([128, 512], F32, tag="res",
                                        name=f"res{b}_{ch}")
                        nc.vector.tensor_mul(res[:, :], AC[b][ch][:, :],
                                             rg[:, :])
                        nc.vector.scalar_tensor_tensor(
                            res[:, :], res[:, :], gb[:, ch:ch + 1],
                            xf_f[:, ch, i0:i0 + 512],
                            op0=add, op1=add)
                        eng = nc.scalar if ch == 1 else nc.sync
                        eng.dma_start(
                            out=out_d[ch * 128:(ch + 1) * 128,
                                      i0:i0 + 512],
                            in_=res[:, :])

    return nc


_CACHE = {}


def _make_in_maps(x, Wq, bq, Wk, bk, Wv, bv, gamma):
    # host-side layout prep (pure relayout, no arithmetic)
    wT = np.concatenate(
        [
            np.tile(np.ascontiguousarray(Wq.T), (1, 4)),
            np.tile(np.ascontiguousarray(Wk.T), (1, 4)),
            np.ascontiguousarray(Wv.T),
        ],
        axis=1,
    ).astype(np.float32)                      # [256, 512]
    bq4 = np.tile(bq, 4).reshape(128, 1).astype(np.float32)
    bk4 = np.tile(bk, 4).reshape(128, 1).astype(np.float32)
    bvP = np.ascontiguousarray(bv.reshape(2, 128).T).astype(np.float32)
    g128 = np.full((128, 1), float(gamma.reshape(-1)[0]), dtype=np.float32)
    cst = np.ascontiguousarray(
        np.concatenate([bq4, bk4, g128, bvP], axis=1))  # [128, 5]

    in_maps = []
    for core in range(8):
        b, half = divmod(core, 2)
        xf = x[b].reshape(C, N)
        # rotate so this core's own half-columns come first: the program
        # is SPMD-uniform (own tokens = columns [0:2048)); attention is
        # permutation-invariant over j
        xp = np.ascontiguousarray(
            np.concatenate([xf[:, half * NH:(half + 1) * NH],
                            xf[:, (1 - half) * NH:(2 - half) * NH]], axis=1))
        in_maps.append({"xf": xp, "wT": wT, "cst": cst})
    return in_maps


def kernel(x, Wq, bq, Wk, bk, Wv, bv, gamma):
    x = np.asarray(x, dtype=np.float32)
    Wq = np.asarray(Wq, dtype=np.float32)
    bq = np.asarray(bq, dtype=np.float32)
    Wk = np.asarray(Wk, dtype=np.float32)
    bk = np.asarray(bk, dtype=np.float32)
    Wv = np.asarray(Wv, dtype=np.float32)
    bv = np.asarray(bv, dtype=np.float32)
    gamma = np.asarray(gamma, dtype=np.float32)

    if "nc" not in _CACHE:
        _CACHE["nc"] = _build_program()
    nc = _CACHE["nc"]

    in_maps = _make_in_maps(x, Wq, bq, Wk, bk, Wv, bv, gamma)
    core_ids = list(range(8))

    from concourse.bass_utils import run_bass_kernel_spmd

    res = run_bass_kernel_spmd(nc, in_maps, core_ids)

    out = np.empty((B, C, N), dtype=np.float32)
    for core in core_ids:
        b, half = divmod(core, 2)
        out[b, :, half * NH:(half + 1) * NH] = res.results[core]["out"]
    return out.reshape(B, C, H, W)


# revision 57
# speedup vs baseline: 1.0436x; 1.0436x over previous
"""Position-attention (SAGAN-style) Bass kernel for 8 Trainium2 NeuronCores.

Reference computation (per batch b, with n = H*W = 4096 spatial tokens):
    q = Wq @ x + bq            [32, n]
    k = Wk @ x + bk            [32, n]
    v = Wv @ x + bv            [256, n]
    att = softmax_j(q_i . k_j) [n, n]
    out = gamma * (v @ att^T) + x

Sharding: 8 cores = 4 batches x 2 token-halves; disjoint outputs, no
collectives. SPMD-uniform program: the host permutes each core's x so its
own 2048 tokens are columns [0:2048) (attention is permutation-invariant
over j, and out/q only touch own columns).

Layout/engine choices:
  - scores^T[j, i] = lhsT(k[d, j_tile]) . rhs(q[d, i]): K=32 contraction,
    4-way PE row-tiling (tile_position=(32t, 0)); q/k built 4x-replicated
    across partitions so row group t finds operands at base partition 32t.
  - rowsum[i] = sum_j e^T[j, i]: M=1 ones-matmuls, 4-way PE column-tiling
    (tile_position=(0, 32c)) -> 4 partials at partitions {0,32,64,96} of
    one PSUM bank. A leading zero-matmul (lhsT=0, M=128) opens the bank:
    one whole-bank has_written clear + zeroes garbage partitions, so the
    4 col-groups can all accumulate with start=False and an all-ones
    K=128 matmul later does combine+broadcast in one shot.
  - 1/rowsum via exp(-ln(rs)) on ScalarE (both funcs live in the
    natural_log_exp_and_others table set -> one ACT_TABLE_LOAD); the DVE
    reciprocal is ~6.3 cyc/elem and was 13.4us of Vector time.
  - out[c, i] = sum_j v^T[j, c] e^T[j, i]: K=128 bf16 matmuls accumulated
    over 32 j-tiles in PSUM (fp8 was evaluated and rejected: quantizing
    e/v to any fp8 format costs 4-7e-2 rel err vs the 2e-2 budget).
  - x loaded once (4MB fp32), split across both HWDGE rings (sync +
    scalar queues), own-block0 chunk first so q/scores start early; bf16
    casts chase the DMAs. No separate xq load (was +2MB).
  - softmax needs no max-subtraction: max score ~25 << 88 (fp32 exp
    overflow), e in bf16. exp(-ln(rs + 1e-30)) keeps gamma=0 exact and
    degenerate rows finite.
Matmul operands are bf16 (fp32 PSUM accumulation); the residual `+ x` is
added in exact fp32.
"""

import os
import sys

for _p in (
    "/root/.axon_site",
    "/root/.axon_site/_ro/trn_rl_repo",
    "/root/.axon_site/_ro/pypackages",
    "/opt/trn_rl_repo",
):
    if os.path.isdir(_p) and _p not in sys.path:
        sys.path.append(_p)

import json

import numpy as np

from concourse import bass, mybir
from concourse.tile import TileContext

F32 = mybir.dt.float32
BF16 = mybir.dt.bfloat16

B, C, H, W = 4, 256, 64, 64
N = H * W            # 4096 tokens
NH = N // 2          # 2048 tokens per core (token half)
MID = C // 8         # 32 qk channels
JT = N // 128        # 32 j-tiles of 128 tokens
NBLK = NH // 512     # 4 i-blocks of 512 tokens per core

SCORE_TP = 4         # score matmul row-tiling ways (4 -> positions 0/32/64/96)
RS_COLS = 4          # rowsum matmul column-tiling ways


def _split_multi_waits(bir_bytes: bytes) -> bytes:
    """Workaround for this container's walrus: it accepts at most ONE sem-wait
    command per lowered instruction ('Too many sync wait commands'), while
    bass/Tile freely attach several. Split extra waits onto preceding NoOps
    on the same engine — per-engine program order makes this semantics-
    preserving (all waits still satisfied before the instruction runs)."""
    d = json.loads(bir_bytes)
    n_split = 0
    for f in d.get("functions", []):
        for bb in f.get("blocks", []):
            out = []
            for ins in bb.get("instructions", []):
                si = ins.get("sync_info")
                waits = si.get("on_wait") if si else None
                if waits and len(waits) > 1:
                    for w in waits[:-1]:
                        n_split += 1
                        out.append(
                            {
                                "debug": ins.get("debug", 0),
                                "engine": ins["engine"],
                                "ins": [],
                                "outs": [],
                                "name": f"{ins['name']}-ws{n_split}",
                                "opcode": "NoOp",
                                "sync_info": {"on_wait": [w], "on_update": []},
                            }
                        )
                    si["on_wait"] = [waits[-1]]
                out.append(ins)
            bb["instructions"] = out
    return json.dumps(d).encode()


_ws_applied = False


def _apply_wait_split_patch():
    global _ws_applied
    if _ws_applied:
        return
    _ws_applied = True
    from concourse import bass_utils, bass2jax

    orig = bass_utils.compile_bir_kernel

    def patched(bir_json, tmpdir, neff_name="file.neff"):
        return orig(_split_multi_waits(bytes(bir_json)), tmpdir, neff_name)

    bass_utils.compile_bir_kernel = patched
    bass2jax.compile_bir_kernel = patched


_apply_wait_split_patch()


def _build_program():
    nc = bass.Bass()

    xf_d = nc.declare_dram_parameter("xf", [C, N], F32, isOutput=False)
    wT_d = nc.declare_dram_parameter("wT", [C, 512], F32, isOutput=False)
    # [bq4 | bk4 | g128 | bvP] packed: one DMA instead of four (each tiny
    # DMA costs ~2us of serial ring time)
    cst_d = nc.declare_dram_parameter("cst", [128, 5], F32, isOutput=False)
    out_d = nc.declare_dram_parameter("out", [C, NH], F32, isOutput=True)

    act = mybir.ActivationFunctionType
    add = mybir.AluOpType.add

    with TileContext(nc) as tc:
        with (
            tc.tile_pool(name="const", bufs=1) as constp,
            tc.tile_pool(name="xf", bufs=1) as xfp,
            tc.tile_pool(name="xb", bufs=1) as xbp,
            tc.tile_pool(name="proj", bufs=1) as projp,
            tc.tile_pool(name="eblk", bufs=2) as eblkp,
            tc.tile_pool(name="small", bufs=2) as smallp,
            tc.tile_pool(name="res", bufs=4) as resp,
            tc.tile_pool(name="psA", bufs=2, space="PSUM") as psA,
            tc.tile_pool(name="psB", bufs=2, space="PSUM") as psB,
            tc.tile_pool(name="psRS", bufs=1, space="PSUM") as psRS,
            tc.tile_pool(name="psBC", bufs=1, space="PSUM") as psBC,
        ):
            # ---- constants / weights ----
            # w split across both HWDGE rings: the q/k half gates the
            # first projections, the v half only the v-projs. Small
            # consts ride the sync ring after its x pieces.
            cst = constp.tile([128, 5], F32, tag="cst")
            nc.gpsimd.dma_start(out=cst[:, :], in_=cst_d[:, :])
            bq4, bk4, g128, bvP = (cst[:, 0:1], cst[:, 1:2], cst[:, 2:3],
                                   cst[:, 3:5])

            w_f = constp.tile([128, 2, 512], F32, tag="wf")
            nc.scalar.dma_start(
                out=w_f[:, :, 0:256],
                in_=wT_d[:, 0:256].rearrange("(two p) n -> p two n",
                                             two=2))
            nc.scalar.dma_start(
                out=w_f[:, :, 256:512],
                in_=wT_d[:, 256:512].rearrange("(two p) n -> p two n",
                                               two=2))
            w_b = constp.tile([128, 2, 512], BF16, tag="wb")
            nc.vector.tensor_copy(w_b[:, :, 0:256], w_f[:, :, 0:256])

            ones_b = constp.tile([128, 1], BF16, tag="ones_b")
            nc.vector.memset(ones_b[:, :], 1.0)
            ones_f = constp.tile([128, 128], F32, tag="ones_f")
            nc.vector.memset(ones_f[:, :], 1.0)
            zero_b = constp.tile([128, 128], BF16, tag="zero_b")
            nc.vector.memset(zero_b[:, :], 0.0)
            eps128 = constp.tile([128, 1], F32, tag="eps")
            nc.vector.memset(eps128[:, :], 1e-30)

            # ---- x loads: own-block0 small chunk first (unblocks q and the
            # first score group), own-rest on the sync ring, other half on
            # the scalar ring so the two HWDGE rings transfer in parallel ----
            xf_f = xfp.tile([128, 2, N], F32, tag="xff")
            x_b = xbp.tile([128, 2, N], BF16, tag="xb")
            pieces = [  # (c0, c1, engine) — pieces 0-1 alone on the sync
                # ring and w alone ahead on the scalar ring, so the
                # critical first transfers don't share HBM bandwidth;
                # later pieces queue behind w on the scalar ring
                (0, 512, nc.sync),
                (512, 1024, nc.sync),
                (1024, 1536, nc.scalar),
                (1536, 2048, nc.scalar),
                (2048, 3072, nc.scalar),
                (3072, 4096, nc.scalar),
            ]
            for c0, c1, eng in pieces:
                eng.dma_start(
                    out=xf_f[:, :, c0:c1],
                    in_=xf_d[:, c0:c1].rearrange("(two p) n -> p two n",
                                                 two=2))
            gb = constp.tile([128, 2], F32, tag="gb")

            def cast_piece(i):
                c0, c1, _ = pieces[i]
                for s0 in range(c0, c1, 512):
                    nc.vector.tensor_copy(x_b[:, :, s0:s0 + 512],
                                          xf_f[:, :, s0:s0 + 512])

            # ---- projections ----
            # q (4x-replicated rows): [128, NH]; own columns only
            q_sb = projp.tile([128, NH], BF16, tag="q")

            def q_proj(ic, pool, tag):
                ps = pool.tile([128, 512], F32, tag=tag, name=f"qp{ic}")
                nc.tensor.matmul(
                    ps[:, :], lhsT=w_b[:, 0, 0:128],
                    rhs=x_b[:, 0, ic * 512:(ic + 1) * 512],
                    start=True, stop=False)
                nc.tensor.matmul(
                    ps[:, :], lhsT=w_b[:, 1, 0:128],
                    rhs=x_b[:, 1, ic * 512:(ic + 1) * 512],
                    start=False, stop=True)
                nc.vector.tensor_scalar_add(
                    q_sb[:, ic * 512:(ic + 1) * 512], ps[:, :], bq4[:, :])

            # k (4x-replicated rows): [128, N]
            k_sb = projp.tile([128, N], BF16, tag="k")

            def k_proj(ic, pool, tag):
                ps = pool.tile([128, 512], F32, tag=tag, name=f"kp{ic}")
                nc.tensor.matmul(
                    ps[:, :], lhsT=w_b[:, 0, 128:256],
                    rhs=x_b[:, 0, ic * 512:(ic + 1) * 512],
                    start=True, stop=False)
                nc.tensor.matmul(
                    ps[:, :], lhsT=w_b[:, 1, 128:256],
                    rhs=x_b[:, 1, ic * 512:(ic + 1) * 512],
                    start=False, stop=True)
                nc.vector.tensor_scalar_add(
                    k_sb[:, ic * 512:(ic + 1) * 512], ps[:, :], bk4[:, :])

            # v^T tiles, flat [128, JT*256]: tile jt at cols [jt*256, +256);
            # two tiles share one PSUM bank so the evacuating cast is a
            # single [128, 512] copy. bv folds into the epilogue.
            v_sb = projp.tile([128, JT * C], BF16, tag="v")

            def v_proj(p, pool, tag):
                ps = pool.tile([128, 512], F32, tag=tag, name=f"vp{p}")
                for t in range(2):
                    jt = 2 * p + t
                    nc.tensor.matmul(
                        ps[:, t * 256:t * 256 + C],
                        lhsT=x_b[:, 0, jt * 128:(jt + 1) * 128],
                        rhs=w_b[:, 0, 256:512], start=True, stop=False)
                    nc.tensor.matmul(
                        ps[:, t * 256:t * 256 + C],
                        lhsT=x_b[:, 1, jt * 128:(jt + 1) * 128],
                        rhs=w_b[:, 1, 256:512], start=False, stop=True)
                nc.vector.tensor_copy(v_sb[:, p * 512:(p + 1) * 512],
                                      ps[:, :])

            # Projection emission plan: the first pieces' q/k go up front
            # through psB, then block 0 opens immediately (its exp stream
            # is the critical path); the remaining projections dribble
            # through BOTH psB (free until block 0's out-accumulators are
            # allocated at group 5) and the psBC bank, in parallel chains,
            # scheduled ahead of their first use. x-casts are emitted just
            # before their first consumer so the in-order DVE queue never
            # parks on a late DMA.
            def pjB(fn, a):
                return lambda: fn(a, psB, "psb")

            def pjC(fn, a):
                return lambda: fn(a, psBC, "psbc")

            proj_sched = {
                0: [lambda: cast_piece(2), pjB(k_proj, 2), pjC(v_proj, 4),
                    pjB(v_proj, 5)],
                1: [lambda: cast_piece(3), pjC(k_proj, 3), pjB(v_proj, 6),
                    pjC(v_proj, 7)],
                2: [lambda: cast_piece(4), pjB(k_proj, 4), pjC(q_proj, 2),
                    pjB(v_proj, 8), pjC(v_proj, 9)],
                3: [lambda: cast_piece(5), pjB(k_proj, 5), pjC(q_proj, 3),
                    pjB(v_proj, 10), pjC(v_proj, 11)],
                4: [pjB(k_proj, 6), pjC(v_proj, 12), pjB(v_proj, 13)],
                5: [pjC(k_proj, 7), pjB(v_proj, 14), pjC(v_proj, 15)],
            }

            # ---- attention blocks: 4 i-blocks of 512 tokens ----
            # Phases fused and software-pipelined: the PE-queue order per
            # group is [scores(g+1) | rowsum(g) | out-acc(g)], so exp(g+1)
            # (ScalarE) overlaps the PE work on group g, and the last
            # block's tail is just one epilogue.
            NG = JT // SCORE_TP
            EB, RS, AC, ASB, RG = {}, {}, {}, {}, {}  # per-block state

            def scores(b, g):
                # one 4-way row-tiled group split across TWO 2-bank psA
                # allocations, so exp(g) reads one pair while the next
                # group's matmuls fill the other: the exp->scores->exp
                # serial chain (psA reuse) never stalls either engine
                i0 = b * 512
                halves = []
                for h in range(2):
                    ph = psA.tile([128, 2, 512], F32, tag="psa",
                                  name=f"ps4_{b}_{g}_{h}")
                    for u in range(2):
                        t = 2 * h + u
                        jt = SCORE_TP * g + t
                        nc.tensor.matmul(
                            ph[:, u, :],
                            lhsT=k_sb[32 * t:32 * (t + 1),
                                      jt * 128:(jt + 1) * 128],
                            rhs=q_sb[32 * t:32 * (t + 1), i0:i0 + 512],
                            start=True, stop=True,
                            tile_position=(32 * t, 0))
                    halves.append(ph)
                return halves

            def emit_exp(e_blk, g, ps4):
                for h, ph in enumerate(ps4):
                    j0 = SCORE_TP * g + 2 * h
                    nc.scalar.activation(
                        e_blk[:, j0:j0 + 2, :], ph[:, :, :], act.Exp)

            def out_group(b, g):
                for ch in range(2):
                    for t in range(SCORE_TP):
                        jt = SCORE_TP * g + t
                        nc.tensor.matmul(
                            AC[b][ch][:, :],
                            lhsT=v_sb[:, jt * 256 + ch * 128:
                                      jt * 256 + ch * 128 + 128],
                            rhs=EB[b][:, jt, :],
                            start=(jt == 0), stop=(jt == JT - 1))

            # out-matmul lag: block 0 defers its out-groups to iterations
            # 5..7 (so psB keeps pipelining projections until then); later
            # blocks lag by one group, which both absorbs the exp->PE
            # semaphore latency and gives the epilogue time to recycle the
            # psB banks across block boundaries.
            B0_OUT = {6: (0, 1, 2), 7: (3, 4, 5, 6)}

            cast_piece(0)
            q_proj(0, psB, "psb")
            k_proj(0, psB, "psb")
            EB[0] = eblkp.tile([128, JT, 512], BF16, tag="e", name="e0")
            ps4 = scores(0, 0)
            emit_exp(EB[0], 0, ps4)
            exp_emitted = True
            nc.vector.tensor_copy(w_b[:, :, 256:512], w_f[:, :, 256:512])
            # gb[c] = gamma * bv[c]  (folded v-bias: out += gamma*bv[c])
            nc.vector.tensor_scalar_mul(gb[:, :], bvP[:, :], g128[:, :])
            v_proj(0, psB, "psb")
            v_proj(1, psB, "psb")
            cast_piece(1)
            k_proj(1, psB, "psb")
            v_proj(2, psB, "psb")
            v_proj(3, psB, "psb")
            q_proj(1, psB, "psb")

            for idx in range(NBLK * NG):
                b, g = divmod(idx, NG)
                e_blk = EB[b]
                if g == 0:
                    # rowsum bank + bank-opener: one whole-bank
                    # has_written clear + zeroes, so the col-tiled
                    # partials all run start=False and the combine matmul
                    # can sum all 128 partitions. Emitted here (not with
                    # the early scores/exp) so it never delays them.
                    rs_ps = RS[b] = psRS.tile([128, 512], F32, tag="psrs",
                                              name=f"rs{b}")
                    nc.tensor.matmul(
                        rs_ps[:, :], lhsT=zero_b[:, :],
                        rhs=x_b[:, 0, 0:512],
                        start=True, stop=False, skip_group_check=True)
                    if b > 0:
                        # previous block's last out-group + accumulator
                        # evacuation + epilogue land HERE, giving the PE
                        # ready work while exp(b,0) runs (kills the
                        # block-boundary bubble)
                        out_group(b - 1, NG - 1)
                        ASB[b - 1] = []
                        for ch in range(2):
                            asb = resp.tile([128, 512], F32, tag="res",
                                            name=f"res{b - 1}_{ch}")
                            nc.vector.tensor_copy(asb[:, :],
                                                  AC[b - 1][ch][:, :])
                            ASB[b - 1].append(asb)
                        AC[b] = [psB.tile([128, 512], F32, tag="psb",
                                          name=f"acc{b}_{ch}")
                                 for ch in range(2)]
                        pi0 = (b - 1) * 512
                        for ch in range(2):
                            res = ASB[b - 1][ch]
                            nc.vector.tensor_mul(res[:, :], res[:, :],
                                                 RG[b - 1][:, :])
                            nc.vector.scalar_tensor_tensor(
                                res[:, :], res[:, :], gb[:, ch:ch + 1],
                                xf_f[:, ch, pi0:pi0 + 512],
                                op0=add, op1=add)
                            nc.sync.dma_start(
                                out=out_d[ch * 128:(ch + 1) * 128,
                                          pi0:pi0 + 512],
                                in_=res[:, :])
                if b == 0 and g == 6:
                    AC[0] = [psB.tile([128, 512], F32, tag="psb",
                                      name=f"acc0_{ch}")
                             for ch in range(2)]
                rs_ps = RS[b]
                if not exp_emitted:
                    emit_exp(e_blk, g, ps4)
                exp_emitted = False
                if g + 1 < NG:
                    ps4 = scores(b, g + 1)
                elif b + 1 < NBLK:
                    # next block's first scores+exp, ahead of this
                    # group's rowsum/out work: ScalarE flows from exp(b,7)
                    # straight into exp(b+1,0) while the PE catches up
                    EB[b + 1] = eblkp.tile([128, JT, 512], BF16, tag="e",
                                           name=f"e{b + 1}")
                    ps4 = scores(b + 1, 0)
                    emit_exp(EB[b + 1], 0, ps4)
                    exp_emitted = True
                for t in range(SCORE_TP):
                    jt = SCORE_TP * g + t
                    c = jt % RS_COLS
                    nc.tensor.matmul(
                        rs_ps[32 * c:32 * c + 1, :],
                        lhsT=ones_b[:, :], rhs=e_blk[:, jt, :],
                        start=False, stop=(jt >= JT - RS_COLS),
                        tile_position=(0, 32 * c),
                        skip_group_check=True)
                if b == 0:
                    for og in B0_OUT.get(g, ()):
                        out_group(0, og)
                    for job in proj_sched.get(g, ()):
                        job()
                elif g >= 1:
                    out_group(b, g - 1)
                if g < NG - 1:
                    continue

                # block end: rowsum partials -> SBUF; all-ones K=128
                # matmul does combine + broadcast to 128 partitions in one
                # shot; 1/rs as exp(-ln(rs)) on ScalarE (both funcs share
                # one ACT table set); * gamma on DVE. The accumulators are
                # evacuated to SBUF immediately (no rg dependency) so the
                # psB banks recycle for block b+1 without waiting on the
                # normalization chain.
                i0 = b * 512
                rs_sb = smallp.tile([128, 512], F32, tag="rs",
                                    name=f"rssb{b}")
                nc.vector.tensor_copy(rs_sb[:, :], rs_ps[:, :])
                bc_ps = psBC.tile([128, 512], F32, tag="psbc",
                                  name=f"bc{b}")
                nc.tensor.matmul(
                    bc_ps[:, :], lhsT=ones_f[:, :], rhs=rs_sb[:, :],
                    start=True, stop=True)
                last = b == NBLK - 1
                if last:
                    out_group(b, NG - 1)
                ln_sb = smallp.tile([128, 512], F32, tag="ln",
                                    name=f"ln{b}")
                nc.scalar.activation(ln_sb[:, :], bc_ps[:, :], act.Ln,
                                     bias=eps128[:, :])
                inv_sb = smallp.tile([128, 512], F32, tag="inv",
                                     name=f"inv{b}")
                nc.scalar.activation(inv_sb[:, :], ln_sb[:, :], act.Exp,
                                     scale=-1.0)
                rg = smallp.tile([128, 512], F32, tag="rg",
                                 name=f"rg{b}")
                nc.vector.tensor_scalar_mul(rg[:, :], inv_sb[:, :],
                                            g128[:, :])
                RG[b] = rg

                # non-last blocks defer their last out-group + epilogue to
                # the next block's first iteration (boundary filler); the
                # last block runs its epilogue straight from PSUM with the
                # two stores on parallel rings to shorten the tail
                if last:
                    for ch in range(2):
                        res = resp.tile([128, 512], F32, tag="res",
                                        name=f"res{b}_{ch}")
                        nc.vector.tensor_mul(res[:, :], AC[b][ch][:, :],
                                             rg[:, :])
                        nc.vector.scalar_tensor_tensor(
                            res[:, :], res[:, :], gb[:, ch:ch + 1],
                            xf_f[:, ch, i0:i0 + 512],
                            op0=add, op1=add)
                        eng = nc.scalar if ch == 1 else nc.sync
                        eng.dma_start(
                            out=out_d[ch * 128:(ch + 1) * 128,
                                      i0:i0 + 512],
                            in_=res[:, :])

    return nc


_CACHE = {}


def _make_in_maps(x, Wq, bq, Wk, bk, Wv, bv, gamma):
    # host-side layout prep (pure relayout, no arithmetic)
    wT = np.concatenate(
        [
            np.tile(np.ascontiguousarray(Wq.T), (1, 4)),
            np.tile(np.ascontiguousarray(Wk.T), (1, 4)),
            np.ascontiguousarray(Wv.T),
        ],
        axis=1,
    ).astype(np.float32)                      # [256, 512]
    bq4 = np.tile(bq, 4).reshape(128, 1).astype(np.float32)
    bk4 = np.tile(bk, 4).reshape(128, 1).astype(np.float32)
    bvP = np.ascontiguousarray(bv.reshape(2, 128).T).astype(np.float32)
    g128 = np.full((128, 1), float(gamma.reshape(-1)[0]), dtype=np.float32)
    cst = np.ascontiguousarray(
        np.concatenate([bq4, bk4, g128, bvP], axis=1))  # [128, 5]

    in_maps = []
    for core in range(8):
        b, half = divmod(core, 2)
        xf = x[b].reshape(C, N)
        # rotate so this core's own half-columns come first: the program
        # is SPMD-uniform (own tokens = columns [0:2048)); attention is
        # permutation-invariant over j
        xp = np.ascontiguousarray(
            np.concatenate([xf[:, half * NH:(half + 1) * NH],
                            xf[:, (1 - half) * NH:(2 - half) * NH]], axis=1))
        in_maps.append({"xf": xp, "wT": wT, "cst": cst})
    return in_maps


def kernel(x, Wq, bq, Wk, bk, Wv, bv, gamma):
    x = np.asarray(x, dtype=np.float32)
    Wq = np.asarray(Wq, dtype=np.float32)
    bq = np.asarray(bq, dtype=np.float32)
    Wk = np.asarray(Wk, dtype=np.float32)
    bk = np.asarray(bk, dtype=np.float32)
    Wv = np.asarray(Wv, dtype=np.float32)
    bv = np.asarray(bv, dtype=np.float32)
    gamma = np.asarray(gamma, dtype=np.float32)

    if "nc" not in _CACHE:
        _CACHE["nc"] = _build_program()
    nc = _CACHE["nc"]

    in_maps = _make_in_maps(x, Wq, bq, Wk, bk, Wv, bv, gamma)
    core_ids = list(range(8))

    from concourse.bass_utils import run_bass_kernel_spmd

    res = run_bass_kernel_spmd(nc, in_maps, core_ids)

    out = np.empty((B, C, N), dtype=np.float32)
    for core in core_ids:
        b, half = divmod(core, 2)
        out[b, :, half * NH:(half + 1) * NH] = res.results[core]["out"]
    return out.reshape(B, C, H, W)


# revision 58
# speedup vs baseline: 1.0484x; 1.0047x over previous
"""Position-attention (SAGAN-style) Bass kernel for 8 Trainium2 NeuronCores.

Reference computation (per batch b, with n = H*W = 4096 spatial tokens):
    q = Wq @ x + bq            [32, n]
    k = Wk @ x + bk            [32, n]
    v = Wv @ x + bv            [256, n]
    att = softmax_j(q_i . k_j) [n, n]
    out = gamma * (v @ att^T) + x

Sharding: 8 cores = 4 batches x 2 token-halves; disjoint outputs, no
collectives. SPMD-uniform program: the host permutes each core's x so its
own 2048 tokens are columns [0:2048) (attention is permutation-invariant
over j, and out/q only touch own columns).

Layout/engine choices:
  - scores^T[j, i] = lhsT(k[d, j_tile]) . rhs(q[d, i]): K=32 contraction,
    4-way PE row-tiling (tile_position=(32t, 0)); q/k built 4x-replicated
    across partitions so row group t finds operands at base partition 32t.
  - rowsum[i] = sum_j e^T[j, i]: M=1 ones-matmuls, 4-way PE column-tiling
    (tile_position=(0, 32c)) -> 4 partials at partitions {0,32,64,96} of
    one PSUM bank. A leading zero-matmul (lhsT=0, M=128) opens the bank:
    one whole-bank has_written clear + zeroes garbage partitions, so the
    4 col-groups can all accumulate with start=False and an all-ones
    K=128 matmul later does combine+broadcast in one shot.
  - 1/rowsum via exp(-ln(rs)) on ScalarE (both funcs live in the
    natural_log_exp_and_others table set -> one ACT_TABLE_LOAD); the DVE
    reciprocal is ~6.3 cyc/elem and was 13.4us of Vector time.
  - out[c, i] = sum_j v^T[j, c] e^T[j, i]: K=128 bf16 matmuls accumulated
    over 32 j-tiles in PSUM (fp8 was evaluated and rejected: quantizing
    e/v to any fp8 format costs 4-7e-2 rel err vs the 2e-2 budget).
  - x loaded once (4MB fp32), split across both HWDGE rings (sync +
    scalar queues), own-block0 chunk first so q/scores start early; bf16
    casts chase the DMAs. No separate xq load (was +2MB).
  - softmax needs no max-subtraction: max score ~25 << 88 (fp32 exp
    overflow), e in bf16. exp(-ln(rs + 1e-30)) keeps gamma=0 exact and
    degenerate rows finite.
Matmul operands are bf16 (fp32 PSUM accumulation); the residual `+ x` is
added in exact fp32.
"""

import os
import sys

for _p in (
    "/root/.axon_site",
    "/root/.axon_site/_ro/trn_rl_repo",
    "/root/.axon_site/_ro/pypackages",
    "/opt/trn_rl_repo",
):
    if os.path.isdir(_p) and _p not in sys.path:
        sys.path.append(_p)

import json

import numpy as np

from concourse import bass, mybir
from concourse.tile import TileContext

F32 = mybir.dt.float32
BF16 = mybir.dt.bfloat16

B, C, H, W = 4, 256, 64, 64
N = H * W            # 4096 tokens
NH = N // 2          # 2048 tokens per core (token half)
MID = C // 8         # 32 qk channels
JT = N // 128        # 32 j-tiles of 128 tokens
NBLK = NH // 512     # 4 i-blocks of 512 tokens per core

SCORE_TP = 4         # score matmul row-tiling ways (4 -> positions 0/32/64/96)
RS_COLS = 4          # rowsum matmul column-tiling ways


def _split_multi_waits(bir_bytes: bytes) -> bytes:
    """Workaround for this container's walrus: it accepts at most ONE sem-wait
    command per lowered instruction ('Too many sync wait commands'), while
    bass/Tile freely attach several. Split extra waits onto preceding NoOps
    on the same engine — per-engine program order makes this semantics-
    preserving (all waits still satisfied before the instruction runs)."""
    d = json.loads(bir_bytes)
    n_split = 0
    for f in d.get("functions", []):
        for bb in f.get("blocks", []):
            out = []
            for ins in bb.get("instructions", []):
                si = ins.get("sync_info")
                waits = si.get("on_wait") if si else None
                if waits and len(waits) > 1:
                    for w in waits[:-1]:
                        n_split += 1
                        out.append(
                            {
                                "debug": ins.get("debug", 0),
                                "engine": ins["engine"],
                                "ins": [],
                                "outs": [],
                                "name": f"{ins['name']}-ws{n_split}",
                                "opcode": "NoOp",
                                "sync_info": {"on_wait": [w], "on_update": []},
                            }
                        )
                    si["on_wait"] = [waits[-1]]
                out.append(ins)
            bb["instructions"] = out
    return json.dumps(d).encode()


_ws_applied = False


def _apply_wait_split_patch():
    global _ws_applied
    if _ws_applied:
        return
    _ws_applied = True
    from concourse import bass_utils, bass2jax

    orig = bass_utils.compile_bir_kernel

    def patched(bir_json, tmpdir, neff_name="file.neff"):
        return orig(_split_multi_waits(bytes(bir_json)), tmpdir, neff_name)

    bass_utils.compile_bir_kernel = patched
    bass2jax.compile_bir_kernel = patched


_apply_wait_split_patch()


def _build_program():
    nc = bass.Bass()

    xf_d = nc.declare_dram_parameter("xf", [C, N], F32, isOutput=False)
    wT_d = nc.declare_dram_parameter("wT", [C, 512], F32, isOutput=False)
    # [bq4 | bk4 | g128 | bvP] packed: one DMA instead of four (each tiny
    # DMA costs ~2us of serial ring time)
    cst_d = nc.declare_dram_parameter("cst", [128, 5], F32, isOutput=False)
    out_d = nc.declare_dram_parameter("out", [C, NH], F32, isOutput=True)

    act = mybir.ActivationFunctionType
    add = mybir.AluOpType.add

    with TileContext(nc) as tc:
        with (
            tc.tile_pool(name="const", bufs=1) as constp,
            tc.tile_pool(name="xf", bufs=1) as xfp,
            tc.tile_pool(name="xb", bufs=1) as xbp,
            tc.tile_pool(name="proj", bufs=1) as projp,
            tc.tile_pool(name="eblk", bufs=2) as eblkp,
            tc.tile_pool(name="small", bufs=2) as smallp,
            tc.tile_pool(name="res", bufs=4) as resp,
            tc.tile_pool(name="psA", bufs=2, space="PSUM") as psA,
            tc.tile_pool(name="psB", bufs=2, space="PSUM") as psB,
            tc.tile_pool(name="psRS", bufs=1, space="PSUM") as psRS,
            tc.tile_pool(name="psBC", bufs=1, space="PSUM") as psBC,
        ):
            # ---- constants / weights ----
            # w split across both HWDGE rings: the q/k half gates the
            # first projections, the v half only the v-projs. Small
            # consts ride the sync ring after its x pieces.
            cst = constp.tile([128, 5], F32, tag="cst")
            nc.gpsimd.dma_start(out=cst[:, :], in_=cst_d[:, :])
            bq4, bk4, g128, bvP = (cst[:, 0:1], cst[:, 1:2], cst[:, 2:3],
                                   cst[:, 3:5])

            w_f = constp.tile([128, 2, 512], F32, tag="wf")
            nc.scalar.dma_start(
                out=w_f[:, :, 0:256],
                in_=wT_d[:, 0:256].rearrange("(two p) n -> p two n",
                                             two=2))
            nc.scalar.dma_start(
                out=w_f[:, :, 256:512],
                in_=wT_d[:, 256:512].rearrange("(two p) n -> p two n",
                                               two=2))
            w_b = constp.tile([128, 2, 512], BF16, tag="wb")
            nc.vector.tensor_copy(w_b[:, :, 0:256], w_f[:, :, 0:256])

            ones_b = constp.tile([128, 1], BF16, tag="ones_b")
            nc.vector.memset(ones_b[:, :], 1.0)
            ones_f = constp.tile([128, 128], F32, tag="ones_f")
            nc.vector.memset(ones_f[:, :], 1.0)
            zero_b = constp.tile([128, 128], BF16, tag="zero_b")
            nc.vector.memset(zero_b[:, :], 0.0)
            eps128 = constp.tile([128, 1], F32, tag="eps")
            nc.vector.memset(eps128[:, :], 1e-30)

            # ---- x loads: own-block0 small chunk first (unblocks q and the
            # first score group), own-rest on the sync ring, other half on
            # the scalar ring so the two HWDGE rings transfer in parallel ----
            xf_f = xfp.tile([128, 2, N], F32, tag="xff")
            x_b = xbp.tile([128, 2, N], BF16, tag="xb")
            pieces = [  # (c0, c1, engine) — pieces 0-1 alone on the sync
                # ring and w alone ahead on the scalar ring, so the
                # critical first transfers don't share HBM bandwidth;
                # later pieces queue behind w on the scalar ring
                (0, 512, nc.sync),
                (512, 1024, nc.sync),
                (1024, 1536, nc.scalar),
                (1536, 2048, nc.scalar),
                (2048, 3072, nc.scalar),
                (3072, 4096, nc.scalar),
            ]
            for c0, c1, eng in pieces:
                eng.dma_start(
                    out=xf_f[:, :, c0:c1],
                    in_=xf_d[:, c0:c1].rearrange("(two p) n -> p two n",
                                                 two=2))
            gb = constp.tile([128, 2], F32, tag="gb")

            def cast_piece(i):
                c0, c1, _ = pieces[i]
                for s0 in range(c0, c1, 512):
                    nc.vector.tensor_copy(x_b[:, :, s0:s0 + 512],
                                          xf_f[:, :, s0:s0 + 512])

            # ---- projections ----
            # q (4x-replicated rows): [128, NH]; own columns only
            q_sb = projp.tile([128, NH], BF16, tag="q")

            def q_proj(ic, pool, tag):
                ps = pool.tile([128, 512], F32, tag=tag, name=f"qp{ic}")
                nc.tensor.matmul(
                    ps[:, :], lhsT=w_b[:, 0, 0:128],
                    rhs=x_b[:, 0, ic * 512:(ic + 1) * 512],
                    start=True, stop=False)
                nc.tensor.matmul(
                    ps[:, :], lhsT=w_b[:, 1, 0:128],
                    rhs=x_b[:, 1, ic * 512:(ic + 1) * 512],
                    start=False, stop=True)
                nc.vector.tensor_scalar_add(
                    q_sb[:, ic * 512:(ic + 1) * 512], ps[:, :], bq4[:, :])

            # k (4x-replicated rows): [128, N]
            k_sb = projp.tile([128, N], BF16, tag="k")

            def k_proj(ic, pool, tag):
                ps = pool.tile([128, 512], F32, tag=tag, name=f"kp{ic}")
                nc.tensor.matmul(
                    ps[:, :], lhsT=w_b[:, 0, 128:256],
                    rhs=x_b[:, 0, ic * 512:(ic + 1) * 512],
                    start=True, stop=False)
                nc.tensor.matmul(
                    ps[:, :], lhsT=w_b[:, 1, 128:256],
                    rhs=x_b[:, 1, ic * 512:(ic + 1) * 512],
                    start=False, stop=True)
                nc.vector.tensor_scalar_add(
                    k_sb[:, ic * 512:(ic + 1) * 512], ps[:, :], bk4[:, :])

            # v^T tiles, flat [128, JT*256]: tile jt at cols [jt*256, +256);
            # two tiles share one PSUM bank so the evacuating cast is a
            # single [128, 512] copy. bv folds into the epilogue.
            v_sb = projp.tile([128, JT * C], BF16, tag="v")

            def v_proj(p, pool, tag):
                ps = pool.tile([128, 512], F32, tag=tag, name=f"vp{p}")
                for t in range(2):
                    jt = 2 * p + t
                    nc.tensor.matmul(
                        ps[:, t * 256:t * 256 + C],
                        lhsT=x_b[:, 0, jt * 128:(jt + 1) * 128],
                        rhs=w_b[:, 0, 256:512], start=True, stop=False)
                    nc.tensor.matmul(
                        ps[:, t * 256:t * 256 + C],
                        lhsT=x_b[:, 1, jt * 128:(jt + 1) * 128],
                        rhs=w_b[:, 1, 256:512], start=False, stop=True)
                nc.vector.tensor_copy(v_sb[:, p * 512:(p + 1) * 512],
                                      ps[:, :])

            # Projection emission plan: the first pieces' q/k go up front
            # through psB, then block 0 opens immediately (its exp stream
            # is the critical path); the remaining projections dribble
            # through BOTH psB (free until block 0's out-accumulators are
            # allocated at group 5) and the psBC bank, in parallel chains,
            # scheduled ahead of their first use. x-casts are emitted just
            # before their first consumer so the in-order DVE queue never
            # parks on a late DMA.
            def pjB(fn, a):
                return lambda: fn(a, psB, "psb")

            def pjC(fn, a):
                return lambda: fn(a, psBC, "psbc")

            proj_sched = {
                0: [lambda: cast_piece(2), pjB(k_proj, 2), pjC(v_proj, 4),
                    pjB(v_proj, 5)],
                1: [lambda: cast_piece(3), pjC(k_proj, 3), pjB(v_proj, 6),
                    pjC(v_proj, 7)],
                2: [lambda: cast_piece(4), pjB(k_proj, 4), pjC(q_proj, 2),
                    pjB(v_proj, 8), pjC(v_proj, 9)],
                3: [lambda: cast_piece(5), pjB(k_proj, 5), pjC(q_proj, 3),
                    pjB(v_proj, 10), pjC(v_proj, 11)],
                4: [pjB(k_proj, 6), pjC(v_proj, 12), pjB(v_proj, 13)],
                5: [pjC(k_proj, 7), pjB(v_proj, 14), pjC(v_proj, 15)],
            }

            # ---- attention blocks: 4 i-blocks of 512 tokens ----
            # Phases fused and software-pipelined: the PE-queue order per
            # group is [scores(g+1) | rowsum(g) | out-acc(g)], so exp(g+1)
            # (ScalarE) overlaps the PE work on group g, and the last
            # block's tail is just one epilogue.
            NG = JT // SCORE_TP
            EB, RS, AC, ASB, RG = {}, {}, {}, {}, {}  # per-block state

            def scores(b, g):
                # one 4-way row-tiled group split across TWO 2-bank psA
                # allocations, so exp(g) reads one pair while the next
                # group's matmuls fill the other: the exp->scores->exp
                # serial chain (psA reuse) never stalls either engine
                i0 = b * 512
                halves = []
                for h in range(2):
                    ph = psA.tile([128, 2, 512], F32, tag="psa",
                                  name=f"ps4_{b}_{g}_{h}")
                    for u in range(2):
                        t = 2 * h + u
                        jt = SCORE_TP * g + t
                        nc.tensor.matmul(
                            ph[:, u, :],
                            lhsT=k_sb[32 * t:32 * (t + 1),
                                      jt * 128:(jt + 1) * 128],
                            rhs=q_sb[32 * t:32 * (t + 1), i0:i0 + 512],
                            start=True, stop=True,
                            tile_position=(32 * t, 0))
                    halves.append(ph)
                return halves

            def emit_exp(e_blk, g, ps4):
                for h, ph in enumerate(ps4):
                    j0 = SCORE_TP * g + 2 * h
                    nc.scalar.activation(
                        e_blk[:, j0:j0 + 2, :], ph[:, :, :], act.Exp)

            def out_group(b, g):
                for ch in range(2):
                    for t in range(SCORE_TP):
                        jt = SCORE_TP * g + t
                        nc.tensor.matmul(
                            AC[b][ch][:, :],
                            lhsT=v_sb[:, jt * 256 + ch * 128:
                                      jt * 256 + ch * 128 + 128],
                            rhs=EB[b][:, jt, :],
                            start=(jt == 0), stop=(jt == JT - 1))

            # out-matmul lag: block 0 defers its out-groups to iterations
            # 5..7 (so psB keeps pipelining projections until then); later
            # blocks lag by one group, which both absorbs the exp->PE
            # semaphore latency and gives the epilogue time to recycle the
            # psB banks across block boundaries.
            B0_OUT = {6: (0, 1, 2), 7: (3, 4, 5, 6)}

            cast_piece(0)
            q_proj(0, psB, "psb")
            k_proj(0, psB, "psb")
            EB[0] = eblkp.tile([128, JT, 512], BF16, tag="e", name="e0")
            ps4 = scores(0, 0)
            emit_exp(EB[0], 0, ps4)
            exp_emitted = True
            nc.vector.tensor_copy(w_b[:, :, 256:512], w_f[:, :, 256:512])
            # gb[c] = gamma * bv[c]  (folded v-bias: out += gamma*bv[c])
            nc.vector.tensor_scalar_mul(gb[:, :], bvP[:, :], g128[:, :])
            v_proj(0, psB, "psb")
            v_proj(1, psB, "psb")
            cast_piece(1)
            k_proj(1, psB, "psb")
            v_proj(2, psB, "psb")
            v_proj(3, psB, "psb")
            q_proj(1, psB, "psb")

            for idx in range(NBLK * NG):
                b, g = divmod(idx, NG)
                e_blk = EB[b]
                if g == 0:
                    # rowsum bank + bank-opener: one whole-bank
                    # has_written clear + zeroes, so the col-tiled
                    # partials all run start=False and the combine matmul
                    # can sum all 128 partitions. Emitted here (not with
                    # the early scores/exp) so it never delays them.
                    rs_ps = RS[b] = psRS.tile([128, 512], F32, tag="psrs",
                                              name=f"rs{b}")
                    # block 0 must zero the whole bank (power-on garbage
                    # could be NaN and the all-ones combine reads every
                    # partition); later blocks reuse the same bank whose
                    # untouched partitions still hold those zeros, so a
                    # 1-column opener (whole-bank has_written clear) is
                    # enough — the col-tiled partials overwrite their own
                    # stale rows on first touch
                    nw = 512 if b == 0 else 1
                    nc.tensor.matmul(
                        rs_ps[:, 0:nw], lhsT=zero_b[:, :],
                        rhs=x_b[:, 0, 0:nw],
                        start=True, stop=False, skip_group_check=True)
                    if b > 0:
                        # previous block's last out-group + accumulator
                        # evacuation + epilogue land HERE, giving the PE
                        # ready work while exp(b,0) runs (kills the
                        # block-boundary bubble)
                        out_group(b - 1, NG - 1)
                        ASB[b - 1] = []
                        for ch in range(2):
                            asb = resp.tile([128, 512], F32, tag="res",
                                            name=f"res{b - 1}_{ch}")
                            nc.vector.tensor_copy(asb[:, :],
                                                  AC[b - 1][ch][:, :])
                            ASB[b - 1].append(asb)
                        AC[b] = [psB.tile([128, 512], F32, tag="psb",
                                          name=f"acc{b}_{ch}")
                                 for ch in range(2)]
                        pi0 = (b - 1) * 512
                        for ch in range(2):
                            res = ASB[b - 1][ch]
                            nc.vector.tensor_mul(res[:, :], res[:, :],
                                                 RG[b - 1][:, :])
                            nc.vector.scalar_tensor_tensor(
                                res[:, :], res[:, :], gb[:, ch:ch + 1],
                                xf_f[:, ch, pi0:pi0 + 512],
                                op0=add, op1=add)
                            nc.sync.dma_start(
                                out=out_d[ch * 128:(ch + 1) * 128,
                                          pi0:pi0 + 512],
                                in_=res[:, :])
                if b == 0 and g == 6:
                    AC[0] = [psB.tile([128, 512], F32, tag="psb",
                                      name=f"acc0_{ch}")
                             for ch in range(2)]
                rs_ps = RS[b]
                if not exp_emitted:
                    emit_exp(e_blk, g, ps4)
                exp_emitted = False
                if g + 1 < NG:
                    ps4 = scores(b, g + 1)
                elif b + 1 < NBLK:
                    # next block's first scores+exp, ahead of this
                    # group's rowsum/out work: ScalarE flows from exp(b,7)
                    # straight into exp(b+1,0) while the PE catches up
                    EB[b + 1] = eblkp.tile([128, JT, 512], BF16, tag="e",
                                           name=f"e{b + 1}")
                    ps4 = scores(b + 1, 0)
                    emit_exp(EB[b + 1], 0, ps4)
                    exp_emitted = True
                for t in range(SCORE_TP):
                    jt = SCORE_TP * g + t
                    c = jt % RS_COLS
                    nc.tensor.matmul(
                        rs_ps[32 * c:32 * c + 1, :],
                        lhsT=ones_b[:, :], rhs=e_blk[:, jt, :],
                        start=False, stop=(jt >= JT - RS_COLS),
                        tile_position=(0, 32 * c),
                        skip_group_check=True)
                if b == 0:
                    for og in B0_OUT.get(g, ()):
                        out_group(0, og)
                    for job in proj_sched.get(g, ()):
                        job()
                elif g >= 1:
                    out_group(b, g - 1)
                if g < NG - 1:
                    continue

                # block end: rowsum partials -> SBUF; all-ones K=128
                # matmul does combine + broadcast to 128 partitions in one
                # shot; 1/rs as exp(-ln(rs)) on ScalarE (both funcs share
                # one ACT table set); * gamma on DVE. The accumulators are
                # evacuated to SBUF immediately (no rg dependency) so the
                # psB banks recycle for block b+1 without waiting on the
                # normalization chain.
                i0 = b * 512
                rs_sb = smallp.tile([128, 512], F32, tag="rs",
                                    name=f"rssb{b}")
                nc.vector.tensor_copy(rs_sb[:, :], rs_ps[:, :])
                bc_ps = psBC.tile([128, 512], F32, tag="psbc",
                                  name=f"bc{b}")
                nc.tensor.matmul(
                    bc_ps[:, :], lhsT=ones_f[:, :], rhs=rs_sb[:, :],
                    start=True, stop=True)
                last = b == NBLK - 1
                if last:
                    out_group(b, NG - 1)
                ln_sb = smallp.tile([128, 512], F32, tag="ln",
                                    name=f"ln{b}")
                nc.scalar.activation(ln_sb[:, :], bc_ps[:, :], act.Ln,
                                     bias=eps128[:, :])
                inv_sb = smallp.tile([128, 512], F32, tag="inv",
                                     name=f"inv{b}")
                nc.scalar.activation(inv_sb[:, :], ln_sb[:, :], act.Exp,
                                     scale=-1.0)
                rg = smallp.tile([128, 512], F32, tag="rg",
                                 name=f"rg{b}")
                nc.vector.tensor_scalar_mul(rg[:, :], inv_sb[:, :],
                                            g128[:, :])
                RG[b] = rg

                # non-last blocks defer their last out-group + epilogue to
                # the next block's first iteration (boundary filler); the
                # last block runs its epilogue straight from PSUM with the
                # two stores on parallel rings to shorten the tail
                if last:
                    for ch in range(2):
                        res = resp.tile([128, 512], F32, tag="res",
                                        name=f"res{b}_{ch}")
                        nc.vector.tensor_mul(res[:, :], AC[b][ch][:, :],
                                             rg[:, :])
                        nc.vector.scalar_tensor_tensor(
                            res[:, :], res[:, :], gb[:, ch:ch + 1],
                            xf_f[:, ch, i0:i0 + 512],
                            op0=add, op1=add)
                        eng = nc.scalar if ch == 1 else nc.sync
                        eng.dma_start(
                            out=out_d[ch * 128:(ch + 1) * 128,
                                      i0:i0 + 512],
                            in_=res[:, :])

    return nc


_CACHE = {}


def _make_in_maps(x, Wq, bq, Wk, bk, Wv, bv, gamma):
    # host-side layout prep (pure relayout, no arithmetic)
    wT = np.concatenate(
        [
            np.tile(np.ascontiguousarray(Wq.T), (1, 4)),
            np.tile(np.ascontiguousarray(Wk.T), (1, 4)),
            np.ascontiguousarray(Wv.T),
        ],
        axis=1,
    ).astype(np.float32)                      # [256, 512]
    bq4 = np.tile(bq, 4).reshape(128, 1).astype(np.float32)
    bk4 = np.tile(bk, 4).reshape(128, 1).astype(np.float32)
    bvP = np.ascontiguousarray(bv.reshape(2, 128).T).astype(np.float32)
    g128 = np.full((128, 1), float(gamma.reshape(-1)[0]), dtype=np.float32)
    cst = np.ascontiguousarray(
        np.concatenate([bq4, bk4, g128, bvP], axis=1))  # [128, 5]

    in_maps = []
    for core in range(8):
        b, half = divmod(core, 2)
        xf = x[b].reshape(C, N)
        # rotate so this core's own half-columns come first: the program
        # is SPMD-uniform (own tokens = columns [0:2048)); attention is
        # permutation-invariant over j
        xp = np.ascontiguousarray(
            np.concatenate([xf[:, half * NH:(half + 1) * NH],
                            xf[:, (1 - half) * NH:(2 - half) * NH]], axis=1))
        in_maps.append({"xf": xp, "wT": wT, "cst": cst})
    return in_maps


def kernel(x, Wq, bq, Wk, bk, Wv, bv, gamma):
    x = np.asarray(x, dtype=np.float32)
    Wq = np.asarray(Wq, dtype=np.float32)
    bq = np.asarray(bq, dtype=np.float32)
    Wk = np.asarray(Wk, dtype=np.float32)
    bk = np.asarray(bk, dtype=np.float32)
    Wv = np.asarray(Wv, dtype=np.float32)
    bv = np.asarray(bv, dtype=np.float32)
    gamma = np.asarray(gamma, dtype=np.float32)

    if "nc" not in _CACHE:
        _CACHE["nc"] = _build_program()
    nc = _CACHE["nc"]

    in_maps = _make_in_maps(x, Wq, bq, Wk, bk, Wv, bv, gamma)
    core_ids = list(range(8))

    from concourse.bass_utils import run_bass_kernel_spmd

    res = run_bass_kernel_spmd(nc, in_maps, core_ids)

    out = np.empty((B, C, N), dtype=np.float32)
    for core in core_ids:
        b, half = divmod(core, 2)
        out[b, :, half * NH:(half + 1) * NH] = res.results[core]["out"]
    return out.reshape(B, C, H, W)
